# revision 3
# baseline (speedup 1.0000x reference)
"""Trainium2 Bass kernel for nn_CausalAggregator.

Computes, for target stocks y:
    out[y, :] = Beta[:, y] @ concat([X, adjacency[:, y, :]], 1) @ W + bias
              = (Beta.T @ X) @ Wf  +  (einsum('ny,nyc->yc', Beta, adj)) @ Wa + bias

Sharding: split Beta / adjacency along the target axis y across 8 cores;
replicate X, weight, bias. Each core computes 512 output rows; no
cross-device reduction.

Per-core algorithm (N=4096 source stocks, Y=512 targets, D=O=256, C=3):
  stream 32 n-tiles of 128 rows:
    G_psum[y_t]  += Beta_tile[:, y_t].T @ X_tile          (PE, K-accum in PSUM)
    prod          = adj_tile * Beta_tile (bcast over c)   (DVE)
    red_psum[r]  += ones[128,1].T @ prod[:, 512-chunk]    (PE partition-reduce)
  epilogue:
    G -> SBUF -> PE-transpose -> GT
    adj_agg flat [1, Y*C] -> strided SBUF-to-SBUF DMA -> adjaggT [3, Y]
    out[y_t] = GT.T @ Wf + adjaggT.T @ Wa  (PSUM accum)  + bias
"""

import math

import numpy as np

import concourse.mybir as mybir
import concourse.tile as tile
from concourse import bacc
from concourse.bass import ds, ts
from concourse.bass_utils import run_bass_kernel_spmd
from concourse.masks import make_identity

P = 128
F32 = mybir.dt.float32

# Full problem shapes (hardcoded; kernel.py must be self-contained).
N_FULL = 4096   # source stocks (contraction axis)
Y_TOTAL = 4096  # target stocks (sharded)
D_FULL = 256    # input features
O_FULL = 256    # output features
C_FULL = 3      # adjacency channels
N_CORES = 8
Y_FULL = Y_TOTAL // N_CORES  # per-core target slice


def emit_causal_agg(tc, io, N, Y, D, O, C):
    nc = tc.nc
    beta, adj, x, w, bias, out = (
        io["beta"], io["adj"], io["x"], io["w"], io["bias"], io["out"])

    n_nt, n_yt, n_dt = N // P, Y // P, D // P
    YC = Y * C
    red_chunks = [min(512, YC - s) for s in range(0, YC, 512)]

    adj_flat = adj.rearrange("n y c -> n (y c)")

    with (
        tc.tile_pool(name="const", bufs=1) as cpool,
        tc.tile_pool(name="io", bufs=4) as iopool,
        tc.tile_pool(name="prod", bufs=3) as ppool,
        tc.tile_pool(name="fin", bufs=1) as fpool,
        tc.tile_pool(name="osb", bufs=2) as opool,
    ):
        # --- constants ---
        identity = cpool.tile([P, P], F32, tag="ident")
        make_identity(nc, identity)
        ones = cpool.tile([P, 1], F32, tag="ones")
        nc.vector.memset(ones, 1.0)
        wf = []
        for d_t in range(n_dt):
            t = cpool.tile([P, O], F32, tag=f"wf{d_t}", name=f"wf{d_t}")
            nc.sync.dma_start(out=t, in_=w[ts(d_t, P), :])
            wf.append(t)
        wa = cpool.tile([C, O], F32, tag="wa")
        nc.sync.dma_start(out=wa, in_=w[D:D + C, :])
        bias_bc = cpool.tile([P, O], F32, tag="bias")
        nc.sync.dma_start(out=bias_bc, in_=bias.unsqueeze(0).to_broadcast((P, O)))

        g_sb = [fpool.tile([P, D], F32, tag=f"gsb{y_t}", name=f"gsb{y_t}")
                for y_t in range(n_yt)]
        red_sb = fpool.tile([1, YC], F32, tag="redsb", name="redsb")

        # --- main streaming loop: PSUM accumulation over n-tiles ---
        with tc.tile_pool(name="acc", bufs=1, space="PSUM") as accpool:
            g_psum = [accpool.tile([P, D], F32, tag=f"g{y_t}", name=f"g{y_t}")
                      for y_t in range(n_yt)]
            red_psum = [accpool.tile([1, sz], F32, tag=f"red{r}", name=f"red{r}")
                        for r, sz in enumerate(red_chunks)]

            for n_t in range(n_nt):
                first, last = n_t == 0, n_t == n_nt - 1
                beta_t = iopool.tile([P, Y], F32, tag="beta")
                nc.sync.dma_start(out=beta_t, in_=beta[ts(n_t, P), :])
                adj_t = iopool.tile([P, YC], F32, tag="adj")
                nc.sync.dma_start(out=adj_t, in_=adj_flat[ts(n_t, P), :])
                x_t = iopool.tile([P, D], F32, tag="x")
                nc.sync.dma_start(out=x_t, in_=x[ts(n_t, P), :])

                for y_t in range(n_yt):
                    nc.tensor.matmul(g_psum[y_t], beta_t[:, ts(y_t, P)], x_t,
                                     start=first, stop=last)

                prod_t = ppool.tile([P, YC], F32, tag="prod")
                nc.vector.tensor_mul(
                    prod_t.rearrange("p (y c) -> p y c", c=C),
                    adj_t.rearrange("p (y c) -> p y c", c=C),
                    beta_t.unsqueeze(2).to_broadcast((P, Y, C)),
                )
                for r, sz in enumerate(red_chunks):
                    nc.tensor.matmul(red_psum[r], ones, prod_t[:, ds(r * 512, sz)],
                                     start=first, stop=last)

            # drain accumulators to SBUF
            for y_t in range(n_yt):
                nc.any.tensor_copy(g_sb[y_t], g_psum[y_t])
            for r, sz in enumerate(red_chunks):
                nc.any.tensor_copy(red_sb[:, ds(r * 512, sz)], red_psum[r])

        # --- epilogue ---
        with tc.tile_pool(name="fpsum", bufs=2, space="PSUM") as fpsum_pool:
            # adj_agg [1, (y c)] -> adjaggT [c, y] via strided SBUF-to-SBUF DMA
            adjaggT = fpool.tile([C, Y], F32, tag="adjaggT")
            red_view = red_sb.rearrange("p (y c) -> p c y", c=C)
            for c in range(C):
                nc.sync.dma_start(out=adjaggT[c:c + 1, :], in_=red_view[:, c, :])

            # GT[d, y] via PE transposes of 128x128 blocks
            gt_sb = [fpool.tile([P, Y], F32, tag=f"gt{d_t}", name=f"gt{d_t}")
                     for d_t in range(n_dt)]
            for y_t in range(n_yt):
                for d_t in range(n_dt):
                    tr = fpsum_pool.tile([P, P], F32, tag="tr")
                    nc.tensor.transpose(tr, g_sb[y_t][:, ts(d_t, P)], identity)
                    nc.any.tensor_copy(gt_sb[d_t][:, ts(y_t, P)], tr)

            # out[y_t] = GT.T @ Wf + adjaggT.T @ Wa + bias
            for y_t in range(n_yt):
                f_psum = fpsum_pool.tile([P, O], F32, tag="fpsum")
                for d_t in range(n_dt):
                    nc.tensor.matmul(f_psum, gt_sb[d_t][:, ts(y_t, P)], wf[d_t],
                                     start=(d_t == 0), stop=False)
                nc.tensor.matmul(f_psum, adjaggT[:, ts(y_t, P)], wa,
                                 start=False, stop=True)
                o_sb = opool.tile([P, O], F32, tag="osb")
                nc.vector.tensor_add(o_sb, f_psum, bias_bc)
                nc.sync.dma_start(out=out[ts(y_t, P), :], in_=o_sb)


def build_nc(N=N_FULL, Y=Y_FULL, D=D_FULL, O=O_FULL, C=C_FULL, reps=1):
    nc = bacc.Bacc("TRN2", target_bir_lowering=False, debug=False)
    io = {
        "beta": nc.dram_tensor("beta", [N, Y], F32, kind="ExternalInput").ap(),
        "adj": nc.dram_tensor("adj", [N, Y, C], F32, kind="ExternalInput").ap(),
        "x": nc.dram_tensor("x", [N, D], F32, kind="ExternalInput").ap(),
        "w": nc.dram_tensor("w", [D + C, O], F32, kind="ExternalInput").ap(),
        "bias": nc.dram_tensor("bias", [O], F32, kind="ExternalInput").ap(),
        "out": nc.dram_tensor("out", [Y, O], F32, kind="ExternalOutput").ap(),
    }
    with tile.TileContext(nc) as tc:
        for _ in range(reps):
            emit_causal_agg(tc, io, N, Y, D, O, C)
    nc.compile()
    return nc


_NC_CACHE = None


def _get_nc():
    global _NC_CACHE
    if _NC_CACHE is None:
        _NC_CACHE = build_nc()
    return _NC_CACHE


def run(adjacency, input_feature, Beta, weight, bias, trace=False):
    nc = _get_nc()
    adjacency = np.asarray(adjacency, dtype=np.float32)
    input_feature = np.ascontiguousarray(np.asarray(input_feature, dtype=np.float32))
    Beta = np.asarray(Beta, dtype=np.float32)
    weight = np.ascontiguousarray(np.asarray(weight, dtype=np.float32))
    bias = np.ascontiguousarray(np.asarray(bias, dtype=np.float32))

    in_maps = []
    for i in range(N_CORES):
        ys = slice(i * Y_FULL, (i + 1) * Y_FULL)
        in_maps.append({
            "beta": np.ascontiguousarray(Beta[:, ys]),
            "adj": np.ascontiguousarray(adjacency[:, ys, :]),
            "x": input_feature,
            "w": weight,
            "bias": bias,
        })
    res = run_bass_kernel_spmd(nc, in_maps, core_ids=list(range(N_CORES)),
                               trace=trace)
    out = np.concatenate([res.results[i]["out"] for i in range(N_CORES)], axis=0)
    return out, res


def kernel(adjacency, input_feature, Beta, weight, bias):
    out, _ = run(adjacency, input_feature, Beta, weight, bias, trace=False)
    return out


# revision 12
# speedup vs baseline: 59.0556x; 59.0556x over previous
"""Trainium2 Bass kernel for nn_CausalAggregator.

Computes, for target stocks y:
    out[y, :] = Beta[:, y] @ concat([X, adjacency[:, y, :]], 1) @ W + bias
              = (Beta.T @ X) @ Wf  +  (einsum('ny,nyc->yc', Beta, adj)) @ Wa + bias

Sharding: split Beta / adjacency along the target axis y across 8 cores;
replicate X, weight, bias. Each core computes 512 output rows; no
cross-device reduction.

Per-core algorithm (N=4096 source stocks, Y=512 targets, D=O=256, C=3).
fp32 matmul streams at 1/4 col/cycle on the PE, so the partition-reduce
for the einsum term is amortized: products are pre-accumulated in groups
of QUAD n-tiles on the DVE (cheap adds), and only the group sums hit the
PE ones-reduce.

  stream 32 n-tiles of 128 rows:
    GT_psum[d_t] += X_tile[:, d_t].T @ Beta_tile        (PE, K-accum in PSUM)
    acc (+)= adj_tile * Beta_tile (bcast over c)        (DVE mul / mul+add)
    every QUAD tiles, for c in 0..C:
      adjaggT_psum[c, :] += ones.T @ acc[:, c::C]       (PE, strided rhs)
  epilogue:
    GT -> SBUF; adjaggT [C, Y] -> SBUF (already transposed)
    out[y_t] = GT.T @ Wf + adjaggT.T @ Wa  (PSUM accum)  + bias
"""

import numpy as np

import concourse.mybir as mybir
import concourse.tile as tile
from concourse import bacc
from concourse.bass import ds, ts
from concourse.bass_utils import run_bass_kernel_spmd

P = 128
F32 = mybir.dt.float32

# Full problem shapes (hardcoded; kernel.py must be self-contained).
N_FULL = 4096   # source stocks (contraction axis)
Y_TOTAL = 4096  # target stocks (sharded)
D_FULL = 256    # input features
O_FULL = 256    # output features
C_FULL = 3      # adjacency channels
N_CORES = 8
Y_FULL = Y_TOTAL // N_CORES  # per-core target slice
QUAD = 4        # n-tiles pre-accumulated on DVE per PE ones-reduce


def emit_causal_agg(tc, io, N, Y, D, O, C,
                    do_g=True, do_mul=True, do_red=True, do_adj_dma=True,
                    do_epilogue=True, quad=QUAD, gp_adds=False, io_bufs=4):
    nc = tc.nc
    beta, adj, x, w, bias, out = (
        io["beta"], io["adj"], io["x"], io["w"], io["bias"], io["out"])

    n_nt, n_yt, n_dt = N // P, Y // P, D // P
    YC = Y * C
    assert Y <= 512, "strided reduce assumes Y <= 512 fp32 moving-operand max"

    adj_flat = adj.rearrange("n y c -> n (y c)")

    with (
        tc.tile_pool(name="const", bufs=1) as cpool,
        tc.tile_pool(name="io", bufs=io_bufs) as iopool,
        tc.tile_pool(name="prod", bufs=3) as ppool,
        tc.tile_pool(name="accg", bufs=2) as apool,
        tc.tile_pool(name="fin", bufs=1) as fpool,
        tc.tile_pool(name="osb", bufs=2) as opool,
    ):
        # --- constants ---
        ones = cpool.tile([P, 1], F32, tag="ones")
        nc.vector.memset(ones, 1.0)
        wf = []
        for d_t in range(n_dt):
            t = cpool.tile([P, O], F32, tag=f"wf{d_t}", name=f"wf{d_t}")
            nc.sync.dma_start(out=t, in_=w[ts(d_t, P), :])
            wf.append(t)
        wa = cpool.tile([C, O], F32, tag="wa")
        nc.sync.dma_start(out=wa, in_=w[D:D + C, :])
        bias_bc = cpool.tile([P, O], F32, tag="bias")
        nc.sync.dma_start(out=bias_bc, in_=bias.unsqueeze(0).to_broadcast((P, O)))

        gt_sb = [fpool.tile([P, Y], F32, tag=f"gt{d_t}", name=f"gt{d_t}")
                 for d_t in range(n_dt)]
        red_sb = fpool.tile([1, YC], F32, tag="redsb", name="redsb")
        adjaggT_sb = fpool.tile([C, Y], F32, tag="adjaggT", name="adjaggT")

        use_red = do_red and do_adj_dma

        # --- main streaming loop: PSUM accumulation over n-tiles ---
        with tc.tile_pool(name="acc", bufs=1, space="PSUM") as accpool:
            gt_psum = [accpool.tile([P, Y], F32, tag=f"gtp{d_t}", name=f"gtp{d_t}")
                       for d_t in range(n_dt)] if do_g else None
            red_chunks = [min(512, YC - s) for s in range(0, YC, 512)]
            red_psum = [accpool.tile([1, sz], F32, tag=f"red{r}", name=f"red{r}")
                        for r, sz in enumerate(red_chunks)] if use_red else None

            acc_t = None
            n_groups = 0
            for n_t in range(n_nt):
                first, last = n_t == 0, n_t == n_nt - 1
                beta_t = iopool.tile([P, Y], F32, tag="beta")
                nc.sync.dma_start(out=beta_t, in_=beta[ts(n_t, P), :])
                if do_adj_dma:
                    adj_t = iopool.tile([P, YC], F32, tag="adj")
                    nc.sync.dma_start(out=adj_t, in_=adj_flat[ts(n_t, P), :])
                x_t = iopool.tile([P, D], F32, tag="x")
                nc.sync.dma_start(out=x_t, in_=x[ts(n_t, P), :])

                if do_g:
                    for d_t in range(n_dt):
                        nc.tensor.matmul(gt_psum[d_t], x_t[:, ts(d_t, P)], beta_t,
                                         start=first, stop=last)

                if do_mul and do_adj_dma:
                    beta_bc = beta_t.unsqueeze(2).to_broadcast((P, Y, C))
                    adj_v = adj_t.rearrange("p (y c) -> p y c", c=C)
                    if n_t % quad == 0:
                        # first tile of the group: acc = adj * beta
                        acc_t = apool.tile([P, YC], F32, tag="accg")
                        nc.vector.tensor_mul(
                            acc_t.rearrange("p (y c) -> p y c", c=C),
                            adj_v, beta_bc)
                    else:
                        prod_t = ppool.tile([P, YC], F32, tag="prod")
                        nc.vector.tensor_mul(
                            prod_t.rearrange("p (y c) -> p y c", c=C),
                            adj_v, beta_bc)
                        if gp_adds and n_t % quad == 2:
                            nc.gpsimd.tensor_add(acc_t, acc_t, prod_t)
                        else:
                            nc.vector.tensor_add(acc_t, acc_t, prod_t)
                    group_done = (n_t % quad == quad - 1) or last
                    if use_red and group_done:
                        for r, sz in enumerate(red_chunks):
                            nc.tensor.matmul(
                                red_psum[r], ones, acc_t[:, ds(r * 512, sz)],
                                start=(n_groups == 0), stop=last)
                        n_groups += 1
                elif use_red and do_adj_dma:
                    # ablation path: reduce raw adj, no multiply
                    for r, sz in enumerate(red_chunks):
                        nc.tensor.matmul(
                            red_psum[r], ones, adj_t[:, ds(r * 512, sz)],
                            start=first, stop=last)

            # drain accumulators to SBUF
            if do_g:
                for d_t in range(n_dt):
                    nc.any.tensor_copy(gt_sb[d_t], gt_psum[d_t])
            else:
                for d_t in range(n_dt):
                    nc.any.memset(gt_sb[d_t], 0.0)
            if use_red:
                for r, sz in enumerate(red_chunks):
                    nc.any.tensor_copy(red_sb[:, ds(r * 512, sz)], red_psum[r])
            else:
                nc.any.memset(red_sb, 0.0)

        # --- epilogue ---
        if not do_epilogue:
            o_sb = opool.tile([P, O], F32, tag="osb")
            nc.any.tensor_copy(o_sb, gt_sb[0][:, :O])
            nc.sync.dma_start(out=out[0:P, :], in_=o_sb)
            return
        with tc.tile_pool(name="fpsum", bufs=2, space="PSUM") as fpsum_pool:
            # adj_agg [1, (y c)] -> adjaggT [c, y] via strided SBUF DMAs
            red_view = red_sb.rearrange("p (y c) -> p c y", c=C)
            for c in range(C):
                nc.sync.dma_start(out=adjaggT_sb[c:c + 1, :], in_=red_view[:, c, :])
            # out[y_t] = GT.T @ Wf + adjaggT.T @ Wa + bias
            for y_t in range(n_yt):
                f_psum = fpsum_pool.tile([P, O], F32, tag="fpsum")
                for d_t in range(n_dt):
                    nc.tensor.matmul(f_psum, gt_sb[d_t][:, ts(y_t, P)], wf[d_t],
                                     start=(d_t == 0), stop=False)
                nc.tensor.matmul(f_psum, adjaggT_sb[:, ts(y_t, P)], wa,
                                 start=False, stop=True)
                o_sb = opool.tile([P, O], F32, tag="osb")
                nc.vector.tensor_add(o_sb, f_psum, bias_bc)
                nc.sync.dma_start(out=out[ts(y_t, P), :], in_=o_sb)


def build_nc(N=N_FULL, Y=Y_FULL, D=D_FULL, O=O_FULL, C=C_FULL, reps=1,
             internal_inputs=False, **flags):
    nc = bacc.Bacc("TRN2", target_bir_lowering=False, debug=False)
    kind = "Internal" if internal_inputs else "ExternalInput"
    io = {
        "beta": nc.dram_tensor("beta", [N, Y], F32, kind=kind).ap(),
        "adj": nc.dram_tensor("adj", [N, Y, C], F32, kind=kind).ap(),
        "x": nc.dram_tensor("x", [N, D], F32, kind=kind).ap(),
        "w": nc.dram_tensor("w", [D + C, O], F32, kind=kind).ap(),
        "bias": nc.dram_tensor("bias", [O], F32, kind=kind).ap(),
        "out": nc.dram_tensor("out", [Y, O], F32, kind="ExternalOutput").ap(),
    }
    with tile.TileContext(nc) as tc:
        for _ in range(reps):
            emit_causal_agg(tc, io, N, Y, D, O, C, **flags)
    nc.compile()
    return nc


_NC_CACHE = None


def _get_nc():
    global _NC_CACHE
    if _NC_CACHE is None:
        _NC_CACHE = build_nc()
    return _NC_CACHE


def run(adjacency, input_feature, Beta, weight, bias, trace=False):
    nc = _get_nc()
    adjacency = np.asarray(adjacency, dtype=np.float32)
    input_feature = np.ascontiguousarray(np.asarray(input_feature, dtype=np.float32))
    Beta = np.asarray(Beta, dtype=np.float32)
    weight = np.ascontiguousarray(np.asarray(weight, dtype=np.float32))
    bias = np.ascontiguousarray(np.asarray(bias, dtype=np.float32))

    in_maps = []
    for i in range(N_CORES):
        ys = slice(i * Y_FULL, (i + 1) * Y_FULL)
        in_maps.append({
            "beta": np.ascontiguousarray(Beta[:, ys]),
            "adj": np.ascontiguousarray(adjacency[:, ys, :]),
            "x": input_feature,
            "w": weight,
            "bias": bias,
        })
    res = run_bass_kernel_spmd(nc, in_maps, core_ids=list(range(N_CORES)),
                               trace=trace)
    out = np.concatenate([res.results[i]["out"] for i in range(N_CORES)], axis=0)
    return out, res


def kernel(adjacency, input_feature, Beta, weight, bias):
    out, _ = run(adjacency, input_feature, Beta, weight, bias, trace=False)
    return out
